# revision 15
# baseline (speedup 1.0000x reference)
"""Trainium2 Bass kernel for ConditionalExpertRouter (dense MoE, all experts).

Math (per reference):
    rh    = relu(condition @ Wr1.T + br1)                  # [B, RH]
    route = softmax(rh @ Wr2.T + br2, axis=-1)             # [B, E]
    h_e   = relu(x @ W1[e].T + b1[e])                      # [B, H]
    y_e   = h_e @ W2[e].T + b2[e]                          # [B, D]
    out   = sum_e route[:, e] * y_e                        # [B, D]

Strategy: data-parallel over B across 8 cores (weights replicated).
On-chip layout is feature-major ("transposed"): activations live as
[feature(partitions), batch(free)] tiles so both expert matmuls contract
along the partition axis with zero on-chip transposes.  The softmax-
weighted sum over experts is folded into the second matmul's PSUM
accumulation: h'_e = relu(h_e) * exp_e (exp replicated across partitions
via a one-hot selector matmul), out_pre = sum_e W2[e].T-matmuls of h'_e
(+ sum_e exp_e*b2[e]), then a single multiply by 1/sum_e exp_e.

Expert matmuls run in bf16 (fp32 accumulation in PSUM); the router runs
in fp32.  Host-side prep does only layout transforms + dtype casts; all
model math happens on-device.
"""

import numpy as np
import ml_dtypes
from contextlib import ExitStack

import concourse.tile as tile
from concourse import bacc, mybir
from concourse.bass_utils import run_bass_kernel_spmd

BF16 = ml_dtypes.bfloat16

# Problem shapes (hardcoded per contract).
B, D, C, E, H, RH = 8192, 1024, 64, 16, 256, 128
NCORES = 8
BS = B // NCORES          # batch rows per core = 1024
NB = 512                  # batch tile (PSUM free-dim limit for fp32)
NBT = BS // NB            # batch tiles per core = 2
P = 128
KD = D // P               # k-tiles over D = 8
HT = H // P               # h-tiles over H = 2
DT = D // P               # d-tiles over D = 8
DG = 2                    # phase-C d-groups (4 PSUM banks each)
DPG = DT // DG            # d-tiles per group = 4

F32 = mybir.dt.float32
BF = mybir.dt.bfloat16
AF = mybir.ActivationFunctionType

_CACHE = {}


def _build():
    nc = bacc.Bacc("TRN2", target_bir_lowering=False, debug=False,
                   enable_asserts=False, num_devices=NCORES)

    # --- DRAM tensors (per-core) ---
    # xtp[p, kt*BS + b] = x[b, kt*128 + p]  (one big-descriptor DMA)
    xtp = nc.dram_tensor("xtp", [P, KD * BS], BF, kind="ExternalInput").ap()
    condt = nc.dram_tensor("condt", [P, BS], F32, kind="ExternalInput").ap()
    # W1 expert-major: w1p[e, p, kt*H + h] = W1[e, h, kt*128 + p]
    w1p = nc.dram_tensor("w1p", [E, P, KD * H], BF, kind="ExternalInput").ap()
    w2p = nc.dram_tensor("w2p", [E, HT, P, D], BF, kind="ExternalInput").ap()
    # aux fp32 pack: [wr1p(128) | wr2t(16) | br1(1) | b1(32) | br2(1)] = 178 cols
    auxp = nc.dram_tensor("auxp", [P, 178], F32, kind="ExternalInput").ap()
    b2p = nc.dram_tensor("b2p", [P, D], BF, kind="ExternalInput").ap()
    # selectors packed in SBUF layout: [128, (E+1)*128]
    selp = nc.dram_tensor("selp", [P, (E + 1) * P], BF, kind="ExternalInput").ap()
    outt = nc.dram_tensor("outt", [D, BS], F32, kind="ExternalOutput").ap()

    with tile.TileContext(nc) as tc, ExitStack() as ctx:
        wp = ctx.enter_context(tc.tile_pool(name="resident", bufs=1))
        w2s = ctx.enter_context(tc.tile_pool(name="w2s", bufs=12))
        hpp = ctx.enter_context(tc.tile_pool(name="hprime", bufs=2))
        work = ctx.enter_context(tc.tile_pool(name="work", bufs=2))
        hrp = ctx.enter_context(tc.tile_pool(name="hrelu", bufs=3))
        outp = ctx.enter_context(tc.tile_pool(name="outs", bufs=4))
        psA = ctx.enter_context(tc.tile_pool(name="psA", bufs=2, space="PSUM"))
        psB = ctx.enter_context(tc.tile_pool(name="psB", bufs=2, space="PSUM"))
        psC = ctx.enter_context(tc.tile_pool(name="psC", bufs=4, space="PSUM"))

        # --- PE clock warm-up ---
        # ~16 throwaway matmuls on scratch data keep the PE busy during the
        # initial DMA loads so the HAM clock gate is already at 8/8 (2.4 GHz)
        # when the real matmul stream starts (saves ~6us of half-rate mms).
        warm = wp.tile([P, NB], BF, tag="warm")
        nc.vector.memset(warm[:], 1.0)
        ps_w = psA.tile([P, NB], F32, tag="pa", name="ps_warm")
        for _ in range(16):
            nc.tensor.matmul(ps_w[:], lhsT=warm[:, 0:P], rhs=warm[:],
                             start=True, stop=True)

        # --- resident loads ---
        # Order matters: small router/aux tensors first (few big-descriptor
        # DMAs), then x, then W1 expert-by-expert so phase B's expert 0 can
        # start a few us in and the W1 stream stays ahead of the PE.
        auxsb = wp.tile([P, 178], F32, tag="aux")
        nc.sync.dma_start(auxsb[:], auxp[:])
        wr1sb = auxsb[:, 0:P]
        wr2sb = auxsb[:, P:P + E]
        br1sb = auxsb[:, P + E:P + E + 1]
        b1sb = auxsb[:, P + E + 1:P + E + 1 + E * HT]
        br2sb = auxsb[:E, P + E + 1 + E * HT:P + E + 2 + E * HT]
        selsb = wp.tile([P, (E + 1) * P], BF, tag="sel")
        nc.sync.dma_start(selsb[:], selp[:])
        condsb = wp.tile([P, BS], F32, tag="cond")
        nc.sync.dma_start(condsb[:], condt[:])
        xtall = wp.tile([P, KD * BS], BF, tag="xt")
        nc.sync.dma_start(xtall[:], xtp[:])
        xtsb = [xtall[:, kt * BS:(kt + 1) * BS] for kt in range(KD)]
        w1sb = []
        hw = KD * P                      # columns per ht half of one expert
        for e in range(E):
            t = wp.tile([P, KD * H], BF, tag=f"w1_{e}", name=f"w1sb{e}")
            for ht in range(HT):
                nc.sync.dma_start(t[:, ht * hw:(ht + 1) * hw],
                                  w1p[e, :, ht * hw:(ht + 1) * hw])
            w1sb.append(t)
        b2sb = wp.tile([P, D], BF, tag="b2")
        nc.sync.dma_start(b2sb[:], b2p[:])

        def sel_ap(s):
            return selsb[:, s * P:(s + 1) * P]

        for bt in range(NBT):
            bsl = slice(bt * NB, (bt + 1) * NB)

            # ---- router ----
            ps_rh = psA.tile([P, NB], F32, tag="pa", name="ps_rh")
            nc.tensor.matmul(ps_rh[:], lhsT=wr1sb[:], rhs=condsb[:, bsl],
                             start=True, stop=True)
            rh_sb = work.tile([P, NB], F32, tag="rh", name="rh_sb")
            nc.scalar.activation(rh_sb[:], ps_rh[:], AF.Relu, bias=br1sb[:, 0:1])
            ps_lg = psA.tile([E, NB], F32, tag="pa", name="ps_lg")
            nc.tensor.matmul(ps_lg[:], lhsT=wr2sb[:], rhs=rh_sb[:],
                             start=True, stop=True)
            # exp(logits + br2) into zero-padded [128, NB] bf16 tile
            expt = work.tile([P, NB], BF, tag="expt", name="expt")
            nc.vector.memset(expt[:], 0.0)
            nc.scalar.activation(expt[:E, :], ps_lg[:], AF.Exp, bias=br2sb[:, 0:1])
            ps_sum = psA.tile([P, NB], F32, tag="pa", name="ps_sum")
            nc.tensor.matmul(ps_sum[:], lhsT=sel_ap(E), rhs=expt[:],
                             start=True, stop=True)
            recip = work.tile([P, NB], F32, tag="recip", name="recip")
            nc.vector.reciprocal(recip[:], ps_sum[:])

            # ---- phase B: h'_e = relu(W1[e] @ x + b1[e]) * exp_e ----
            hp_big = hpp.tile([P, E * HT * NB], BF, tag="hp", name="hp_big")
            for e in range(E):
                ps_rep = psA.tile([P, NB], F32, tag="pa", name=f"ps_rep{e}")
                nc.tensor.matmul(ps_rep[:], lhsT=sel_ap(e), rhs=expt[:],
                                 start=True, stop=True)
                for ht in range(HT):
                    j = e * HT + ht
                    ps_h = psB.tile([P, NB], F32, tag="ph", name=f"ps_h{j}")
                    for kt in range(KD):
                        col = (ht * KD + kt) * P
                        nc.tensor.matmul(ps_h[:],
                                         lhsT=w1sb[e][:, col:col + P],
                                         rhs=xtsb[kt][:, bsl],
                                         start=(kt == 0), stop=(kt == KD - 1))
                    hr = hrp.tile([P, NB], BF, tag="hr", name=f"hr{j}")
                    nc.scalar.activation(hr[:], ps_h[:], AF.Relu,
                                         bias=b1sb[:, j:j + 1])
                    nc.vector.tensor_mul(hp_big[:, j * NB:(j + 1) * NB],
                                         hr[:], ps_rep[:])

            # ---- phase C: out_pre[dt] = sum_e W2[e].T @ h'_e (+ exp*b2) ----
            for dg in range(DG):
                accs = []
                for i in range(DPG):
                    dt = dg * DPG + i
                    pa = psC.tile([P, NB], F32, tag="cacc", name=f"acc{dt}")
                    nc.tensor.matmul(pa[:], lhsT=b2sb[:, dt * P:(dt + 1) * P],
                                     rhs=expt[:], start=True, stop=False)
                    accs.append(pa)
                for e in range(E):
                    for ht in range(HT):
                        j = e * HT + ht
                        w2t = w2s.tile([P, DPG * P], BF, tag="w2t",
                                       name=f"w2t{dg}_{j}")
                        nc.sync.dma_start(
                            w2t[:], w2p[e][ht][:, dg * DPG * P:(dg + 1) * DPG * P])
                        last = (e == E - 1 and ht == HT - 1)
                        for i in range(DPG):
                            nc.tensor.matmul(accs[i][:],
                                             lhsT=w2t[:, i * P:(i + 1) * P],
                                             rhs=hp_big[:, j * NB:(j + 1) * NB],
                                             start=False, stop=last)
                for i in range(DPG):
                    dt = dg * DPG + i
                    osb = outp.tile([P, NB], F32, tag="ot", name=f"ot{dt}")
                    nc.vector.tensor_mul(osb[:], accs[i][:], recip[:])
                    nc.sync.dma_start(outt[dt * P:(dt + 1) * P, bsl], osb[:])

    nc.compile()
    return nc


def _prep_shared(W1, b1, W2, b2, Wr1, br1, Wr2, br2):
    """Host-side layout transforms + casts for the (core-replicated) weights."""
    # w1p[e, p, (ht*KD + kt)*P + hh] = W1[e, ht*P + hh, kt*P + p]
    # (ht-major so each expert's W1 streams in per-ht halves)
    w1p = np.ascontiguousarray(
        W1.reshape(E, HT, P, KD, P).transpose(0, 4, 1, 3, 2)
        .reshape(E, P, KD * H)).astype(BF16)
    w2p = np.ascontiguousarray(
        W2.transpose(0, 2, 1).reshape(E, HT, P, D)).astype(BF16)
    # aux pack: [wr1p(128) | wr2t(16) | br1(1) | b1(32) | br2(1)]
    aux = np.zeros((P, 178), np.float32)
    aux[:C, 0:P] = Wr1.T                         # [C, RH], zero-padded K
    aux[:, P:P + E] = Wr2.T                      # [RH, E]
    aux[:, P + E] = br1                          # [RH]
    aux[:, P + E + 1:P + E + 1 + E * HT] = (
        b1.reshape(E, HT, P).transpose(2, 0, 1).reshape(P, E * HT))
    aux[:E, P + E + 1 + E * HT] = br2            # [E]
    b2p = np.zeros((P, D), BF16)
    b2p[:E, :] = b2.astype(BF16)
    selp = np.zeros((P, (E + 1) * P), BF16)
    for e in range(E):
        selp[e, e * P:(e + 1) * P] = 1.0         # broadcast-row selector
    selp[:E, E * P:(E + 1) * P] = 1.0            # sum-over-experts selector
    return dict(w1p=w1p, w2p=w2p, auxp=aux, b2p=b2p, selp=selp)


LAST_RESULTS = None


def kernel(x, condition, W1, b1, W2, b2, Wr1, br1, Wr2, br2):
    global LAST_RESULTS
    if "nc" not in _CACHE:
        _CACHE["nc"] = _build()
    nc = _CACHE["nc"]

    shared = _prep_shared(W1, b1, W2, b2, Wr1, br1, Wr2, br2)
    xT = np.ascontiguousarray(x.astype(np.float32).T)        # [D, B]
    condT = np.zeros((P, B), np.float32)
    condT[:C, :] = condition.T

    in_maps = []
    for c in range(NCORES):
        sl = slice(c * BS, (c + 1) * BS)
        m = dict(shared)
        # xtp[p, kt*BS + b] = xT[kt*128 + p, b]
        m["xtp"] = np.ascontiguousarray(
            xT[:, sl].reshape(KD, P, BS).transpose(1, 0, 2).reshape(P, KD * BS)
        ).astype(BF16)
        m["condt"] = np.ascontiguousarray(condT[:, sl])
        in_maps.append(m)

    res = run_bass_kernel_spmd(nc, in_maps, core_ids=list(range(NCORES)))
    LAST_RESULTS = res

    out = np.empty((B, D), np.float32)
    for c in range(NCORES):
        out[c * BS:(c + 1) * BS, :] = res.results[c]["outt"].T
    return out


# revision 16
# speedup vs baseline: 1.0030x; 1.0030x over previous
"""Trainium2 Bass kernel for ConditionalExpertRouter (dense MoE, all experts).

Math (per reference):
    rh    = relu(condition @ Wr1.T + br1)                  # [B, RH]
    route = softmax(rh @ Wr2.T + br2, axis=-1)             # [B, E]
    h_e   = relu(x @ W1[e].T + b1[e])                      # [B, H]
    y_e   = h_e @ W2[e].T + b2[e]                          # [B, D]
    out   = sum_e route[:, e] * y_e                        # [B, D]

Strategy: data-parallel over B across 8 cores (weights replicated).
On-chip layout is feature-major ("transposed"): activations live as
[feature(partitions), batch(free)] tiles so both expert matmuls contract
along the partition axis with zero on-chip transposes.  The softmax-
weighted sum over experts is folded into the second matmul's PSUM
accumulation: h'_e = relu(h_e) * exp_e (exp replicated across partitions
via a one-hot selector matmul), out_pre = sum_e W2[e].T-matmuls of h'_e
(+ sum_e exp_e*b2[e]), then a single multiply by 1/sum_e exp_e.

Expert matmuls run in bf16 (fp32 accumulation in PSUM); the router runs
in fp32.  Host-side prep does only layout transforms + dtype casts; all
model math happens on-device.
"""

import numpy as np
import ml_dtypes
from contextlib import ExitStack

import concourse.tile as tile
from concourse import bacc, mybir
from concourse.bass_utils import run_bass_kernel_spmd

BF16 = ml_dtypes.bfloat16

# Problem shapes (hardcoded per contract).
B, D, C, E, H, RH = 8192, 1024, 64, 16, 256, 128
NCORES = 8
BS = B // NCORES          # batch rows per core = 1024
NB = 512                  # batch tile (PSUM free-dim limit for fp32)
NBT = BS // NB            # batch tiles per core = 2
P = 128
KD = D // P               # k-tiles over D = 8
HT = H // P               # h-tiles over H = 2
DT = D // P               # d-tiles over D = 8
DG = 2                    # phase-C d-groups (4 PSUM banks each)
DPG = DT // DG            # d-tiles per group = 4

F32 = mybir.dt.float32
BF = mybir.dt.bfloat16
AF = mybir.ActivationFunctionType

_CACHE = {}


def _build():
    nc = bacc.Bacc("TRN2", target_bir_lowering=False, debug=False,
                   enable_asserts=False, num_devices=NCORES)

    # --- DRAM tensors (per-core) ---
    # xtp[p, kt*BS + b] = x[b, kt*128 + p]  (one big-descriptor DMA)
    xtp = nc.dram_tensor("xtp", [P, KD * BS], BF, kind="ExternalInput").ap()
    condt = nc.dram_tensor("condt", [P, BS], F32, kind="ExternalInput").ap()
    # W1 expert-major: w1p[e, p, kt*H + h] = W1[e, h, kt*128 + p]
    w1p = nc.dram_tensor("w1p", [E, P, KD * H], BF, kind="ExternalInput").ap()
    w2p = nc.dram_tensor("w2p", [E, HT, P, D], BF, kind="ExternalInput").ap()
    # aux fp32 pack: [wr1p(128) | wr2t(16) | br1(1) | b1(32) | br2(1)] = 178 cols
    auxp = nc.dram_tensor("auxp", [P, 178], F32, kind="ExternalInput").ap()
    b2p = nc.dram_tensor("b2p", [P, D], BF, kind="ExternalInput").ap()
    # selectors packed in SBUF layout: [128, (E+1)*128]
    selp = nc.dram_tensor("selp", [P, (E + 1) * P], BF, kind="ExternalInput").ap()
    outt = nc.dram_tensor("outt", [D, BS], F32, kind="ExternalOutput").ap()

    with tile.TileContext(nc) as tc, ExitStack() as ctx:
        wp = ctx.enter_context(tc.tile_pool(name="resident", bufs=1))
        w2s = ctx.enter_context(tc.tile_pool(name="w2s", bufs=12))
        hpp = ctx.enter_context(tc.tile_pool(name="hprime", bufs=2))
        work = ctx.enter_context(tc.tile_pool(name="work", bufs=2))
        hrp = ctx.enter_context(tc.tile_pool(name="hrelu", bufs=3))
        outp = ctx.enter_context(tc.tile_pool(name="outs", bufs=4))
        psA = ctx.enter_context(tc.tile_pool(name="psA", bufs=2, space="PSUM"))
        psB = ctx.enter_context(tc.tile_pool(name="psB", bufs=2, space="PSUM"))
        psC = ctx.enter_context(tc.tile_pool(name="psC", bufs=4, space="PSUM"))

        # --- PE clock warm-up ---
        # ~16 throwaway matmuls on scratch data keep the PE busy during the
        # initial DMA loads so the HAM clock gate is already at 8/8 (2.4 GHz)
        # when the real matmul stream starts (saves ~6us of half-rate mms).
        warm = wp.tile([P, NB], BF, tag="warm")
        nc.vector.memset(warm[:], 1.0)
        ps_w = psA.tile([P, NB], F32, tag="pa", name="ps_warm")
        for _ in range(16):
            nc.tensor.matmul(ps_w[:], lhsT=warm[:, 0:P], rhs=warm[:],
                             start=True, stop=True)

        # --- resident loads ---
        # Order matters: small router/aux tensors first (few big-descriptor
        # DMAs), then x, then W1 expert-by-expert so phase B's expert 0 can
        # start a few us in and the W1 stream stays ahead of the PE.
        auxsb = wp.tile([P, 178], F32, tag="aux")
        nc.sync.dma_start(auxsb[:], auxp[:])
        wr1sb = auxsb[:, 0:P]
        wr2sb = auxsb[:, P:P + E]
        br1sb = auxsb[:, P + E:P + E + 1]
        b1sb = auxsb[:, P + E + 1:P + E + 1 + E * HT]
        br2sb = auxsb[:E, P + E + 1 + E * HT:P + E + 2 + E * HT]
        selsb = wp.tile([P, (E + 1) * P], BF, tag="sel")
        nc.sync.dma_start(selsb[:], selp[:])
        condsb = wp.tile([P, BS], F32, tag="cond")
        nc.sync.dma_start(condsb[:], condt[:])
        xtall = wp.tile([P, KD * BS], BF, tag="xt")
        for kt in range(KD):
            nc.sync.dma_start(xtall[:, kt * BS:(kt + 1) * BS],
                              xtp[:, kt * BS:(kt + 1) * BS])
        xtsb = [xtall[:, kt * BS:(kt + 1) * BS] for kt in range(KD)]
        w1sb = []
        hw = KD * P                      # columns per ht half of one expert
        for e in range(E):
            t = wp.tile([P, KD * H], BF, tag=f"w1_{e}", name=f"w1sb{e}")
            for ht in range(HT):
                nc.sync.dma_start(t[:, ht * hw:(ht + 1) * hw],
                                  w1p[e, :, ht * hw:(ht + 1) * hw])
            w1sb.append(t)
        b2sb = wp.tile([P, D], BF, tag="b2")
        nc.sync.dma_start(b2sb[:], b2p[:])

        def sel_ap(s):
            return selsb[:, s * P:(s + 1) * P]

        for bt in range(NBT):
            bsl = slice(bt * NB, (bt + 1) * NB)

            # ---- router ----
            ps_rh = psA.tile([P, NB], F32, tag="pa", name="ps_rh")
            nc.tensor.matmul(ps_rh[:], lhsT=wr1sb[:], rhs=condsb[:, bsl],
                             start=True, stop=True)
            rh_sb = work.tile([P, NB], F32, tag="rh", name="rh_sb")
            nc.scalar.activation(rh_sb[:], ps_rh[:], AF.Relu, bias=br1sb[:, 0:1])
            ps_lg = psA.tile([E, NB], F32, tag="pa", name="ps_lg")
            nc.tensor.matmul(ps_lg[:], lhsT=wr2sb[:], rhs=rh_sb[:],
                             start=True, stop=True)
            # exp(logits + br2) into zero-padded [128, NB] bf16 tile
            expt = work.tile([P, NB], BF, tag="expt", name="expt")
            nc.vector.memset(expt[:], 0.0)
            nc.scalar.activation(expt[:E, :], ps_lg[:], AF.Exp, bias=br2sb[:, 0:1])
            ps_sum = psA.tile([P, NB], F32, tag="pa", name="ps_sum")
            nc.tensor.matmul(ps_sum[:], lhsT=sel_ap(E), rhs=expt[:],
                             start=True, stop=True)
            recip = work.tile([P, NB], F32, tag="recip", name="recip")
            nc.vector.reciprocal(recip[:], ps_sum[:])

            # ---- phase B: h'_e = relu(W1[e] @ x + b1[e]) * exp_e ----
            hp_big = hpp.tile([P, E * HT * NB], BF, tag="hp", name="hp_big")
            for e in range(E):
                ps_rep = psA.tile([P, NB], F32, tag="pa", name=f"ps_rep{e}")
                nc.tensor.matmul(ps_rep[:], lhsT=sel_ap(e), rhs=expt[:],
                                 start=True, stop=True)
                for ht in range(HT):
                    j = e * HT + ht
                    ps_h = psB.tile([P, NB], F32, tag="ph", name=f"ps_h{j}")
                    for kt in range(KD):
                        col = (ht * KD + kt) * P
                        nc.tensor.matmul(ps_h[:],
                                         lhsT=w1sb[e][:, col:col + P],
                                         rhs=xtsb[kt][:, bsl],
                                         start=(kt == 0), stop=(kt == KD - 1))
                    hr = hrp.tile([P, NB], BF, tag="hr", name=f"hr{j}")
                    nc.scalar.activation(hr[:], ps_h[:], AF.Relu,
                                         bias=b1sb[:, j:j + 1])
                    nc.vector.tensor_mul(hp_big[:, j * NB:(j + 1) * NB],
                                         hr[:], ps_rep[:])

            # ---- phase C: out_pre[dt] = sum_e W2[e].T @ h'_e (+ exp*b2) ----
            for dg in range(DG):
                accs = []
                for i in range(DPG):
                    dt = dg * DPG + i
                    pa = psC.tile([P, NB], F32, tag="cacc", name=f"acc{dt}")
                    nc.tensor.matmul(pa[:], lhsT=b2sb[:, dt * P:(dt + 1) * P],
                                     rhs=expt[:], start=True, stop=False)
                    accs.append(pa)
                for e in range(E):
                    for ht in range(HT):
                        j = e * HT + ht
                        w2t = w2s.tile([P, DPG * P], BF, tag="w2t",
                                       name=f"w2t{dg}_{j}")
                        nc.sync.dma_start(
                            w2t[:], w2p[e][ht][:, dg * DPG * P:(dg + 1) * DPG * P])
                        last = (e == E - 1 and ht == HT - 1)
                        for i in range(DPG):
                            nc.tensor.matmul(accs[i][:],
                                             lhsT=w2t[:, i * P:(i + 1) * P],
                                             rhs=hp_big[:, j * NB:(j + 1) * NB],
                                             start=False, stop=last)
                for i in range(DPG):
                    dt = dg * DPG + i
                    osb = outp.tile([P, NB], F32, tag="ot", name=f"ot{dt}")
                    nc.vector.tensor_mul(osb[:], accs[i][:], recip[:])
                    nc.sync.dma_start(outt[dt * P:(dt + 1) * P, bsl], osb[:])

    nc.compile()
    return nc


def _prep_shared(W1, b1, W2, b2, Wr1, br1, Wr2, br2):
    """Host-side layout transforms + casts for the (core-replicated) weights."""
    # w1p[e, p, (ht*KD + kt)*P + hh] = W1[e, ht*P + hh, kt*P + p]
    # (ht-major so each expert's W1 streams in per-ht halves)
    w1p = np.ascontiguousarray(
        W1.reshape(E, HT, P, KD, P).transpose(0, 4, 1, 3, 2)
        .reshape(E, P, KD * H)).astype(BF16)
    w2p = np.ascontiguousarray(
        W2.transpose(0, 2, 1).reshape(E, HT, P, D)).astype(BF16)
    # aux pack: [wr1p(128) | wr2t(16) | br1(1) | b1(32) | br2(1)]
    aux = np.zeros((P, 178), np.float32)
    aux[:C, 0:P] = Wr1.T                         # [C, RH], zero-padded K
    aux[:, P:P + E] = Wr2.T                      # [RH, E]
    aux[:, P + E] = br1                          # [RH]
    aux[:, P + E + 1:P + E + 1 + E * HT] = (
        b1.reshape(E, HT, P).transpose(2, 0, 1).reshape(P, E * HT))
    aux[:E, P + E + 1 + E * HT] = br2            # [E]
    b2p = np.zeros((P, D), BF16)
    b2p[:E, :] = b2.astype(BF16)
    selp = np.zeros((P, (E + 1) * P), BF16)
    for e in range(E):
        selp[e, e * P:(e + 1) * P] = 1.0         # broadcast-row selector
    selp[:E, E * P:(E + 1) * P] = 1.0            # sum-over-experts selector
    return dict(w1p=w1p, w2p=w2p, auxp=aux, b2p=b2p, selp=selp)


LAST_RESULTS = None


def kernel(x, condition, W1, b1, W2, b2, Wr1, br1, Wr2, br2):
    global LAST_RESULTS
    if "nc" not in _CACHE:
        _CACHE["nc"] = _build()
    nc = _CACHE["nc"]

    shared = _prep_shared(W1, b1, W2, b2, Wr1, br1, Wr2, br2)
    xT = np.ascontiguousarray(x.astype(np.float32).T)        # [D, B]
    condT = np.zeros((P, B), np.float32)
    condT[:C, :] = condition.T

    in_maps = []
    for c in range(NCORES):
        sl = slice(c * BS, (c + 1) * BS)
        m = dict(shared)
        # xtp[p, kt*BS + b] = xT[kt*128 + p, b]
        m["xtp"] = np.ascontiguousarray(
            xT[:, sl].reshape(KD, P, BS).transpose(1, 0, 2).reshape(P, KD * BS)
        ).astype(BF16)
        m["condt"] = np.ascontiguousarray(condT[:, sl])
        in_maps.append(m)

    res = run_bass_kernel_spmd(nc, in_maps, core_ids=list(range(NCORES)))
    LAST_RESULTS = res

    out = np.empty((B, D), np.float32)
    for c in range(NCORES):
        out[c * BS:(c + 1) * BS, :] = res.results[c]["outt"].T
    return out
